# revision 80
# baseline (speedup 1.0000x reference)
"""MultiHeadAttention kernel for 8x TRN2 NeuronCores.

The reference module's einsum reduces the attention tensor over BOTH the
query and key axes (attn_mass = sum_{q,k} softmax(logits)_k), and softmax
rows sum to 1, so attn_mass == Lq exactly for every (batch, head). The
whole computation therefore collapses to

    out = (Lq * (V_heads @ Wv^T + bv)).reshape(N, L, E) @ Wo^T + bo

which is a single dense GEMM after folding the (block-diagonal) per-head
V-projection into the output projection:

    out = V_flat @ W_eff + b_eff
    W_eff[h*hd+a, n] = Lq * sum_b Wv[b, a] * Wo[n, h*hd+b]      (1024 x 1024)
    b_eff[n]         = Lq * sum_{h,b} Wo[n, h*hd+b] * bv[b] + bo[n]

The device kernel is the GEMM in bf16 (the correctness gate is 2e-2
rel-err; bf16 lands ~2.6e-3), row-sharded across 8 cores (512 rows per
core), computed in TRANSPOSED orientation: out^T[n, m] = sum_k W[k, n]
X[m, k].  Each PSUM bank j holds output columns j*128..(j+1)*128 on
partitions x all 512 rows on the free dim, accumulating lhsT = W-block j
(natural layout) against rhs = X^T k-slabs.

Schedule (from NTFF trace analysis; measured constants in comments):
  * fixed costs per launch: ~6.5us BSP/framework preamble before user
    DMAs can issue, ~1.5-2.5us DMA-queue startup after issue, and a
    ~8.8us NEFF close (walrus's 253-semaphore reset storm + final
    barrier) -- none of it controllable from kernel code;
  * matmul full rate is 216ns per [128x128]x[128x512]; the PE p-state
    reaches 2.4 GHz only after ~3-4us of UNINTERRUPTED activity and
    a stall resets it, so short [128,128] warm-up matmuls on memset
    data bridge the preamble -> first-data gap exactly;
  * only the sync + scalar HWDGE queues are used (gpsimd's soft-DGE
    queue collapses HW-queue throughput while active); per-queue rate
    ramps from ~100-250 B/ns to 300-390 B/ns with >=2 KiB lines;
  * the PE gate (w0 k-slabs 0-3 + X slabs 0-3, 0.625 MB) is ONE fused
    transfer leading the sync queue; X slabs 4-7 follow on the same
    queue so their arrival (T0+1.7us) tracks bank 1's k4 deadline
    (T0+1.73us) at any uniform rate; w1 leads the scalar queue and
    lands before T0; all later W blocks have multi-us margin;
  * emission: bank 0's k0-3 partial first (smallest gate), banks 1-7
    full (evenly-spaced completions whose evictions + output DMAs
    overlap the stream), then bank 0's k4-7 finish in a 3/4 chunk
    plus a 1/4-wide full-k chunk that BORROWS bank 1's long-evicted
    PSUM, so the true tail is one small eviction + a 32 KB DMA;
  * bias varies along PARTITIONS -> folded into the PSUM eviction as a
    per-partition tensor_scalar_add on the vector engine, which also
    casts fp32 PSUM to the bf16 output (separate o tiles per chunk --
    a shared tile makes Tile serialize behind output-DMA delivery).

The host transposes V-shards in, and the (E, RPC) per-core outputs back.
"""

import numpy as np
import ml_dtypes

import concourse.bass as bass
import concourse.bacc as bacc
import concourse.mybir as mybir
from concourse.tile import TileContext
from concourse.bass_utils import run_bass_kernel_spmd

N_CORES = 8
E = 1024            # embed dim == d_model
H, HD = 16, 64      # heads, head dim
ROWS = 4096         # N * L = 2 * 2048
RPC = ROWS // N_CORES   # rows per core = 512
P = 128             # SBUF partitions
KT = E // P         # 8 contraction slabs
JT = E // P         # 8 output-column banks
N_WARM = 37         # bf16 warm-up matmuls bridging preamble -> first data

_NC_CACHE = {}
LAST_RESULTS = None  # BassKernelResults of the most recent device run


def _build(dtype, n_warm=N_WARM):
    f32 = mybir.dt.float32
    odt = f32 if dtype == mybir.dt.float32r else dtype
    nc = bacc.Bacc(None, target_bir_lowering=False)
    # PE-gating data fused into ONE DMA per queue (one issue slot, one
    # completion semaphore): wxa = [w0 | X^T slabs 0-3] on sync,
    # xbw = [X^T slabs 4-7 | bias(bf16)] on scalar.  Fat >=4 KiB lines
    # run ~300+ GB/s/queue once the DMA path has ramped.
    wxa = nc.declare_dram_parameter("wxa", [P, E // 2 + 4 * RPC], dtype,
                                    isOutput=False)
    w0b = nc.declare_dram_parameter("w0b", [P, E // 2], dtype, isOutput=False)
    xbw = nc.declare_dram_parameter("xbw", [P, 4 * RPC], dtype,
                                    isOutput=False)
    bw = nc.declare_dram_parameter("bw", [P, JT], f32, isOutput=False)
    ws = [None] + [
        nc.declare_dram_parameter(f"w{j}", [P, E], dtype, isOutput=False)
        for j in range(1, JT)
    ]
    outT = nc.declare_dram_parameter("outT", [E, RPC], odt, isOutput=True)

    with TileContext(nc) as tc:
        with (
            tc.tile_pool(name="xp", bufs=1) as xp,
            tc.tile_pool(name="wp", bufs=1) as wp,
            tc.tile_pool(name="bp", bufs=1) as bp,
            tc.tile_pool(name="pp", bufs=1, space="PSUM") as pp,
            tc.tile_pool(name="op", bufs=1) as op,
        ):
            # memset needs no DMA: warm-up matmuls can start right after
            # the BSP preamble, well before any input data lands.
            wm_t = bp.tile([P, P], dtype, name="wm", tag="wm")
            nc.gpsimd.memset(wm_t[:], 1.0)

            wxa_t = xp.tile([P, E // 2 + 4 * RPC], dtype, name="wxa",
                            tag="wxa")
            w0b_t = xp.tile([P, E // 2], dtype, name="w0b", tag="w0b")
            xbw_t = xp.tile([P, 4 * RPC], dtype, name="xbw", tag="xbw")
            bw_t = bp.tile([P, JT], f32, name="bw", tag="bw")
            wts = [None] + [
                wp.tile([P, E], dtype, name=f"w{j}", tag=f"w{j}")
                for j in range(1, JT)
            ]

            def w0s(k):
                if k < 4:
                    return wxa_t[:, k * P:(k + 1) * P]
                return w0b_t[:, (k - 4) * P:(k - 3) * P]

            def bias(j):
                return bw_t[:, j:j + 1]

            # Queue plan.  Measured behavior: the DMA path ramps over the
            # first ~3us (both HW queues ~100-250 B/ns early, 300-390
            # after); first packets ~1.5-2.5us after issue; each
            # dma_start costs ~0.6-0.8us of issue time on its engine.
            # gpsimd's software-DGE queue STRANGLES the HW queues while
            # active (measured 3x collapse) -- never use it alongside
            # the critical stream.
            # Rate-correlated feed: the PE-gating wxa (w0 k-slabs 0-3 +
            # X slabs 0-3, 0.625 MB) and then xbw (X slabs 4-7) both
            # ride the EARLIER-starting sync queue back to back.  Banks
            # 0-3 all run k0-3 first, so xbw's deadline is T0 + 3.46us
            # against an arrival of T0 + ~1.7-3us even on slow runs.
            # w1-w3 lead the scalar queue (deadlines T0+0.86/1.73/2.6).
            #   sync:   wxa, xbw, w0b, w5, w7      (+ even outputs)
            #   scalar: w1, bw, w2, w3, w4, w6     (+ odd outputs)
            nc.sync.dma_start(out=wxa_t[:], in_=wxa[:, :])
            nc.scalar.dma_start(out=wts[1][:], in_=ws[1][:, :])
            nc.sync.dma_start(out=xbw_t[:], in_=xbw[:, :])
            nc.scalar.dma_start(out=bw_t[:], in_=bw[:, :])
            nc.scalar.dma_start(out=wts[2][:], in_=ws[2][:, :])
            nc.sync.dma_start(out=w0b_t[:], in_=w0b[:, :])
            nc.scalar.dma_start(out=wts[3][:], in_=ws[3][:, :])
            nc.scalar.dma_start(out=wts[4][:], in_=ws[4][:, :])
            nc.sync.dma_start(out=wts[5][:], in_=ws[5][:, :])
            nc.scalar.dma_start(out=wts[6][:], in_=ws[6][:, :])
            nc.sync.dma_start(out=wts[7][:], in_=ws[7][:, :])

            q3 = 3 * RPC // 4
            ps = [
                pp.tile([P, RPC], f32, name=f"ps{j}", tag=f"ps{j}")
                for j in range(JT)
            ]

            # Low-activity bf16 PE warm-up on nonzero memset data,
            # bridging the preamble -> first-data gap so the PE stays
            # continuously busy (p-state ramps to 2.4 GHz only after
            # ~3-4us of uninterrupted activity; a stall resets it).
            # NOTE: full-width warm-ups measured ~4us SLOWER overall
            # (HAM activity budget) -- keep them 128 wide.
            for i in range(n_warm):
                nc.tensor.matmul(
                    ps[i % JT][:, 0:P],
                    wm_t[:, 0:P],
                    wm_t[:, 0:P],
                    start=True,
                    stop=True,
                )

            def rhs(k):
                if k < 4:
                    return wxa_t[:, E // 2 + k * RPC:E // 2 + (k + 1) * RPC]
                return xbw_t[:, (k - 4) * RPC:(k - 3) * RPC]

            # Emission order: bank 0's k0-3 partial first (its gate is
            # only the 0.625 MB wxa transfer), then banks 1-7 in full,
            # then bank 0's k4-7 finish.  Banks complete ~evenly spread
            # so their evictions + output DMAs overlap the stream; the
            # LAST eviction is split 3/4 + 1/4 with SEPARATE o tiles (a
            # shared tile makes Tile serialize the second eviction
            # behind the first chunk's output DMA delivery).
            def emit(j, ks, stop_at_end):
                first = ks[0] == 0
                for k in ks:
                    nc.tensor.matmul(
                        ps[j] if j != 0 else ps[0][:, 0:q3],
                        wts[j][:, k * P:(k + 1) * P] if j != 0 else w0s(k),
                        rhs(k) if j != 0 else rhs(k)[:, 0:q3],
                        start=(k == 0),
                        stop=(stop_at_end and k == ks[-1]),
                    )

            def evict(j):
                o = op.tile([P, RPC], odt, name=f"o{j}", tag=f"o{j}")
                oq = nc.sync if j % 2 == 0 else nc.scalar
                nc.vector.tensor_scalar_add(o[:], ps[j], bias(j))
                oq.dma_start(out=outT[j * P:(j + 1) * P, :], in_=o[:])

            # Banks 0-3 all run k0-3 first: xbw's PE deadline becomes
            # T0 + 3.46us (vs T0 + 1.73 with only bank 0 split), which
            # covers its arrival even on HBM-contended runs.
            for j in range(4):
                emit(j, list(range(4)), stop_at_end=False)
            for j in range(1, 4):
                emit(j, list(range(4, KT)), stop_at_end=True)
                evict(j)
            for j in range(4, JT):
                emit(j, list(range(KT)), stop_at_end=True)
                evict(j)
            # bank 0 chunk A (cols 0:q3): finish k4-7 and evict.
            for k in range(4, KT):
                nc.tensor.matmul(
                    ps[0][:, 0:q3],
                    w0s(k),
                    rhs(k)[:, 0:q3],
                    start=False,
                    stop=(k == KT - 1),
                )
            oa = op.tile([P, q3], odt, name="o0a", tag="o0a")
            nc.vector.tensor_scalar_add(oa[:], ps[0][:, 0:q3], bias(0))
            nc.sync.dma_start(out=outT[0:P, 0:q3], in_=oa[:])
            # bank 0 chunk B (last RPC-q3 cols): full k0-7 accumulation
            # at the very end, BORROWING bank 1's psum (long evicted).
            # Its matmuls overlap chunk A's eviction; the small final
            # eviction + 32 KB output DMA are the only true tail.
            for k in range(KT):
                nc.tensor.matmul(
                    ps[1][:, 0:RPC - q3],
                    w0s(k),
                    rhs(k)[:, q3:RPC],
                    start=(k == 0),
                    stop=(k == KT - 1),
                )
            ob = op.tile([P, RPC - q3], odt, name="o0b", tag="o0b")
            nc.vector.tensor_scalar_add(ob[:], ps[1][:, 0:RPC - q3], bias(0))
            # split the final 32 KB output across BOTH queues: the two
            # ~0.6us DMA-issue instructions run on separate engines in
            # parallel, and the two 16 KB deliveries overlap.
            hb = (RPC - q3) // 2
            nc.scalar.dma_start(out=outT[0:P, q3:q3 + hb], in_=ob[:, 0:hb])
            nc.sync.dma_start(out=outT[0:P, q3 + hb:RPC], in_=ob[:, hb:])
    nc.compile()
    return nc


def _get_nc(dtype_name, n_warm=N_WARM):
    key = (dtype_name, n_warm)
    if key not in _NC_CACHE:
        _NC_CACHE[key] = _build(getattr(mybir.dt, dtype_name), n_warm)
    return _NC_CACHE[key]


def _prep_in_maps(V, Wv, bv, Wo, bo, lq, np_dtype):
    V = np.ascontiguousarray(np.asarray(V, dtype=np.float32))
    Wv64 = np.asarray(Wv, np.float64)
    Wo64 = np.asarray(Wo, np.float64)
    bv64 = np.asarray(bv, np.float64)
    bo64 = np.asarray(bo, np.float64)

    # Fold per-head V-projection + output projection + attention mass (== Lq).
    Wo_r = Wo64.reshape(E, H, HD)                       # [n, h, b]
    W_eff = lq * np.einsum("ba,nhb->han", Wv64, Wo_r, optimize=True)
    W_eff = W_eff.reshape(E, E).astype(np.float32)      # [k, n]
    b_eff = (lq * np.einsum("nhb,b->n", Wo_r, bv64) + bo64).astype(np.float32)

    # wc[j*P + p, k*P + c] = W_eff[k*P + p, j*P + c]  (lhsT blocks, natural)
    wc = np.ascontiguousarray(
        W_eff.reshape(KT, P, JT, P).transpose(2, 1, 0, 3).reshape(JT * P, E)
    ).astype(np_dtype)
    bw_blk = np.ascontiguousarray(b_eff.reshape(JT, P).T)  # [p, j] fp32

    wmap = {
        f"w{j}": np.ascontiguousarray(wc[j * P:(j + 1) * P, :])
        for j in range(1, JT)
    }

    X = V.reshape(ROWS, E).astype(np_dtype)
    in_maps = []
    for i in range(N_CORES):
        xs_i = X[i * RPC:(i + 1) * RPC, :].T.reshape(KT, P, RPC)
        m = dict(wmap)
        # chunk layout: X^T slab c occupies columns c*RPC..(c+1)*RPC
        xa = xs_i[0:4].transpose(1, 0, 2).reshape(P, 4 * RPC)
        xb = xs_i[4:8].transpose(1, 0, 2).reshape(P, 4 * RPC)
        wxa_i = np.empty((P, E // 2 + 4 * RPC), np_dtype)
        wxa_i[:, :E // 2] = wc[0:P, 0:E // 2]
        wxa_i[:, E // 2:] = xa
        m["wxa"] = wxa_i
        m["w0b"] = np.ascontiguousarray(wc[0:P, E // 2:])
        m["xbw"] = np.ascontiguousarray(xb)
        m["bw"] = bw_blk
        in_maps.append(m)
    return in_maps


def kernel(Q, K, V, Wq, bq, Wk, bk, Wv, bv, Wo, bo, dtype_name="bfloat16",
           n_warm=N_WARM, **_unused):
    global LAST_RESULTS
    n, L, e = np.asarray(V).shape
    lq = float(np.asarray(Q).shape[1])
    np_dtype = (np.dtype(ml_dtypes.bfloat16) if dtype_name == "bfloat16"
                else np.float32)
    in_maps = _prep_in_maps(V, Wv, bv, Wo, bo, lq, np_dtype)
    nc = _get_nc(dtype_name, n_warm)
    LAST_RESULTS = run_bass_kernel_spmd(nc, in_maps, list(range(N_CORES)))
    out = np.concatenate(
        [LAST_RESULTS.results[i]["outT"].T.astype(np.float32)
         for i in range(N_CORES)],
        axis=0,
    )
    return np.ascontiguousarray(out).reshape(n, L, E)


# revision 86
# speedup vs baseline: 1.0269x; 1.0269x over previous
"""MultiHeadAttention kernel for 8x TRN2 NeuronCores.

The reference module's einsum reduces the attention tensor over BOTH the
query and key axes (attn_mass = sum_{q,k} softmax(logits)_k), and softmax
rows sum to 1, so attn_mass == Lq exactly for every (batch, head). The
whole computation therefore collapses to

    out = (Lq * (V_heads @ Wv^T + bv)).reshape(N, L, E) @ Wo^T + bo

which is a single dense GEMM after folding the (block-diagonal) per-head
V-projection into the output projection:

    out = V_flat @ W_eff + b_eff
    W_eff[h*hd+a, n] = Lq * sum_b Wv[b, a] * Wo[n, h*hd+b]      (1024 x 1024)
    b_eff[n]         = Lq * sum_{h,b} Wo[n, h*hd+b] * bv[b] + bo[n]

The device kernel is the GEMM in bf16 (the correctness gate is 2e-2
rel-err; bf16 lands ~2.6e-3), row-sharded across 8 cores (512 rows per
core), computed in TRANSPOSED orientation: out^T[n, m] = sum_k W[k, n]
X[m, k].  Each PSUM bank j holds output columns j*128..(j+1)*128 on
partitions x all 512 rows on the free dim, accumulating lhsT = W-block j
(natural layout) against rhs = X^T k-slabs.

Schedule (from NTFF trace analysis; measured constants in comments):
  * fixed costs per launch: ~6.5us BSP/framework preamble before user
    DMAs can issue, ~1.5-2.5us DMA-queue startup after issue, and a
    ~8.8us NEFF close (walrus's 253-semaphore reset storm + final
    barrier) -- none of it controllable from kernel code;
  * matmul full rate is 216ns per [128x128]x[128x512]; the PE p-state
    reaches 2.4 GHz only after ~3-4us of UNINTERRUPTED activity and
    a stall resets it, so short [128,128] warm-up matmuls on memset
    data bridge the preamble -> first-data gap exactly;
  * only the sync + scalar HWDGE queues are used (gpsimd's soft-DGE
    queue collapses HW-queue throughput while active); per-queue rate
    ramps from ~100-250 B/ns to 300-390 B/ns with >=2 KiB lines;
  * the PE gate (w0 k-slabs 0-3 + X slabs 0-3, 0.625 MB) is ONE fused
    transfer leading the sync queue; X slabs 4-7 follow on the same
    queue so their arrival (T0+1.7us) tracks bank 1's k4 deadline
    (T0+1.73us) at any uniform rate; w1 leads the scalar queue and
    lands before T0; all later W blocks have multi-us margin;
  * emission: bank 0's k0-3 partial first (smallest gate), banks 1-7
    full (evenly-spaced completions whose evictions + output DMAs
    overlap the stream), then bank 0's k4-7 finish in a 3/4 chunk
    plus a 1/4-wide full-k chunk that BORROWS bank 1's long-evicted
    PSUM, so the true tail is one small eviction + a 32 KB DMA;
  * bias varies along PARTITIONS -> folded into the PSUM eviction as a
    per-partition tensor_scalar_add on the vector engine, which also
    casts fp32 PSUM to the bf16 output (separate o tiles per chunk --
    a shared tile makes Tile serialize behind output-DMA delivery).

The host transposes V-shards in, and the (E, RPC) per-core outputs back.
"""

import numpy as np
import ml_dtypes

import concourse.bass as bass
import concourse.bacc as bacc
import concourse.mybir as mybir
from concourse.tile import TileContext
from concourse.bass_utils import run_bass_kernel_spmd

N_CORES = 8
E = 1024            # embed dim == d_model
H, HD = 16, 64      # heads, head dim
ROWS = 4096         # N * L = 2 * 2048
RPC = ROWS // N_CORES   # rows per core = 512
P = 128             # SBUF partitions
KT = E // P         # 8 contraction slabs
JT = E // P         # 8 output-column banks
N_WARM = 37         # bf16 warm-up matmuls bridging preamble -> first data

_NC_CACHE = {}
LAST_RESULTS = None  # BassKernelResults of the most recent device run


def _build(dtype, n_warm=N_WARM):
    f32 = mybir.dt.float32
    odt = f32 if dtype == mybir.dt.float32r else dtype
    nc = bacc.Bacc(None, target_bir_lowering=False)
    # PE-gating data fused into ONE DMA per queue (one issue slot, one
    # completion semaphore): wxa = [w0 | X^T slabs 0-3] on sync,
    # xbw = [X^T slabs 4-7 | bias(bf16)] on scalar.  Fat >=4 KiB lines
    # run ~300+ GB/s/queue once the DMA path has ramped.
    wxa = nc.declare_dram_parameter("wxa", [P, E // 2 + 4 * RPC], dtype,
                                    isOutput=False)
    w0b = nc.declare_dram_parameter("w0b", [P, E // 2], dtype, isOutput=False)
    xbw = nc.declare_dram_parameter("xbw", [P, 3 * RPC], dtype,
                                    isOutput=False)
    x7 = nc.declare_dram_parameter("x7", [P, RPC], dtype, isOutput=False)
    bw = nc.declare_dram_parameter("bw", [P, JT], f32, isOutput=False)
    ws = [None] + [
        nc.declare_dram_parameter(f"w{j}", [P, E], dtype, isOutput=False)
        for j in range(1, JT)
    ]
    outT = nc.declare_dram_parameter("outT", [E, RPC], odt, isOutput=True)

    with TileContext(nc) as tc:
        with (
            tc.tile_pool(name="xp", bufs=1) as xp,
            tc.tile_pool(name="wp", bufs=1) as wp,
            tc.tile_pool(name="bp", bufs=1) as bp,
            tc.tile_pool(name="pp", bufs=1, space="PSUM") as pp,
            tc.tile_pool(name="op", bufs=1) as op,
        ):
            # memset needs no DMA: warm-up matmuls can start right after
            # the BSP preamble, well before any input data lands.
            wm_t = bp.tile([P, P], dtype, name="wm", tag="wm")
            nc.gpsimd.memset(wm_t[:], 1.0)

            wxa_t = xp.tile([P, E // 2 + 4 * RPC], dtype, name="wxa",
                            tag="wxa")
            w0b_t = xp.tile([P, E // 2], dtype, name="w0b", tag="w0b")
            xbw_t = xp.tile([P, 3 * RPC], dtype, name="xbw", tag="xbw")
            x7_t = xp.tile([P, RPC], dtype, name="x7", tag="x7")
            bw_t = bp.tile([P, JT], f32, name="bw", tag="bw")
            wts = [None] + [
                wp.tile([P, E], dtype, name=f"w{j}", tag=f"w{j}")
                for j in range(1, JT)
            ]

            def w0s(k):
                if k < 4:
                    return wxa_t[:, k * P:(k + 1) * P]
                return w0b_t[:, (k - 4) * P:(k - 3) * P]

            def bias(j):
                return bw_t[:, j:j + 1]

            # Queue plan.  Measured behavior: the DMA path ramps over the
            # first ~3us (both HW queues ~100-250 B/ns early, 300-390
            # after); first packets ~1.5-2.5us after issue; each
            # dma_start costs ~0.6-0.8us of issue time on its engine.
            # gpsimd's software-DGE queue STRANGLES the HW queues while
            # active (measured 3x collapse) -- never use it alongside
            # the critical stream.
            # Rate-correlated feed: the PE-gating wxa (w0 k-slabs 0-3 +
            # X slabs 0-3, 0.625 MB) and then xbw (X slabs 4-7) both
            # ride the EARLIER-starting sync queue back to back.  Banks
            # 0-3 all run k0-3 first, so xbw's deadline is T0 + 3.46us
            # against an arrival of T0 + ~1.7-3us even on slow runs.
            # w1-w3 lead the scalar queue (deadlines T0+0.86/1.73/2.6).
            # X slab 7 peels off to the scalar queue so the sync-queue
            # chase (xbw = slabs 4-6 only) lands at T0+1.25 against its
            # T0+1.73 deadline, while slab 7's deadline is T0+2.4
            # against a ~T0 arrival on scalar.
            #   sync:   wxa, xbw(4-6), w0b, w5, w7  (+ even outputs)
            #   scalar: w1, x7, bw, w2, w3, w4, w6  (+ odd outputs)
            nc.sync.dma_start(out=wxa_t[:], in_=wxa[:, :])
            nc.scalar.dma_start(out=wts[1][:], in_=ws[1][:, :])
            nc.sync.dma_start(out=xbw_t[:], in_=xbw[:, :])
            nc.scalar.dma_start(out=x7_t[:], in_=x7[:, :])
            nc.scalar.dma_start(out=bw_t[:], in_=bw[:, :])
            nc.scalar.dma_start(out=wts[2][:], in_=ws[2][:, :])
            nc.sync.dma_start(out=w0b_t[:], in_=w0b[:, :])
            nc.scalar.dma_start(out=wts[3][:], in_=ws[3][:, :])
            nc.scalar.dma_start(out=wts[4][:], in_=ws[4][:, :])
            nc.sync.dma_start(out=wts[5][:], in_=ws[5][:, :])
            nc.scalar.dma_start(out=wts[6][:], in_=ws[6][:, :])
            nc.sync.dma_start(out=wts[7][:], in_=ws[7][:, :])

            q3 = 3 * RPC // 4
            ps = [
                pp.tile([P, RPC], f32, name=f"ps{j}", tag=f"ps{j}")
                for j in range(JT)
            ]

            # Low-activity bf16 PE warm-up on nonzero memset data,
            # bridging the preamble -> first-data gap so the PE stays
            # continuously busy (p-state ramps to 2.4 GHz only after
            # ~3-4us of uninterrupted activity; a stall resets it).
            # NOTE: full-width warm-ups measured ~4us SLOWER overall
            # (HAM activity budget) -- keep them 128 wide.
            for i in range(n_warm):
                nc.tensor.matmul(
                    ps[i % JT][:, 0:P],
                    wm_t[:, 0:P],
                    wm_t[:, 0:P],
                    start=True,
                    stop=True,
                )

            def rhs(k):
                if k < 4:
                    return wxa_t[:, E // 2 + k * RPC:E // 2 + (k + 1) * RPC]
                if k < 7:
                    return xbw_t[:, (k - 4) * RPC:(k - 3) * RPC]
                return x7_t[:, :]

            # Emission order: bank 0's k0-3 partial first (its gate is
            # only the 0.625 MB wxa transfer), then banks 1-7 in full,
            # then bank 0's k4-7 finish.  Banks complete ~evenly spread
            # so their evictions + output DMAs overlap the stream; the
            # LAST eviction is split 3/4 + 1/4 with SEPARATE o tiles (a
            # shared tile makes Tile serialize the second eviction
            # behind the first chunk's output DMA delivery).
            def emit(j, ks, stop_at_end):
                first = ks[0] == 0
                for k in ks:
                    nc.tensor.matmul(
                        ps[j] if j != 0 else ps[0][:, 0:q3],
                        wts[j][:, k * P:(k + 1) * P] if j != 0 else w0s(k),
                        rhs(k) if j != 0 else rhs(k)[:, 0:q3],
                        start=(k == 0),
                        stop=(stop_at_end and k == ks[-1]),
                    )

            def evict(j):
                o = op.tile([P, RPC], odt, name=f"o{j}", tag=f"o{j}")
                oq = nc.sync if j % 2 == 0 else nc.scalar
                nc.vector.tensor_scalar_add(o[:], ps[j], bias(j))
                oq.dma_start(out=outT[j * P:(j + 1) * P, :], in_=o[:])

            emit(0, list(range(4)), stop_at_end=False)
            for j in range(1, JT):
                emit(j, list(range(KT)), stop_at_end=True)
                evict(j)
            # bank 0 chunk A (cols 0:q3): finish k4-7 and evict.
            for k in range(4, KT):
                nc.tensor.matmul(
                    ps[0][:, 0:q3],
                    w0s(k),
                    rhs(k)[:, 0:q3],
                    start=False,
                    stop=(k == KT - 1),
                )
            oa = op.tile([P, q3], odt, name="o0a", tag="o0a")
            nc.vector.tensor_scalar_add(oa[:], ps[0][:, 0:q3], bias(0))
            nc.sync.dma_start(out=outT[0:P, 0:q3], in_=oa[:])
            # bank 0 chunk B (last RPC-q3 cols): full k0-7 accumulation
            # at the very end, BORROWING bank 1's psum (long evicted).
            # Its matmuls overlap chunk A's eviction; the small final
            # eviction + 32 KB output DMA are the only true tail.
            for k in range(KT):
                nc.tensor.matmul(
                    ps[1][:, 0:RPC - q3],
                    w0s(k),
                    rhs(k)[:, q3:RPC],
                    start=(k == 0),
                    stop=(k == KT - 1),
                )
            ob = op.tile([P, RPC - q3], odt, name="o0b", tag="o0b")
            nc.vector.tensor_scalar_add(ob[:], ps[1][:, 0:RPC - q3], bias(0))
            # split the final 32 KB output across BOTH queues: the two
            # ~0.6us DMA-issue instructions run on separate engines in
            # parallel, and the two 16 KB deliveries overlap.
            hb = (RPC - q3) // 2
            nc.scalar.dma_start(out=outT[0:P, q3:q3 + hb], in_=ob[:, 0:hb])
            nc.sync.dma_start(out=outT[0:P, q3 + hb:RPC], in_=ob[:, hb:])
    nc.compile()
    return nc


def _get_nc(dtype_name, n_warm=N_WARM):
    key = (dtype_name, n_warm)
    if key not in _NC_CACHE:
        _NC_CACHE[key] = _build(getattr(mybir.dt, dtype_name), n_warm)
    return _NC_CACHE[key]


def _prep_in_maps(V, Wv, bv, Wo, bo, lq, np_dtype):
    V = np.ascontiguousarray(np.asarray(V, dtype=np.float32))
    Wv64 = np.asarray(Wv, np.float64)
    Wo64 = np.asarray(Wo, np.float64)
    bv64 = np.asarray(bv, np.float64)
    bo64 = np.asarray(bo, np.float64)

    # Fold per-head V-projection + output projection + attention mass (== Lq).
    Wo_r = Wo64.reshape(E, H, HD)                       # [n, h, b]
    W_eff = lq * np.einsum("ba,nhb->han", Wv64, Wo_r, optimize=True)
    W_eff = W_eff.reshape(E, E).astype(np.float32)      # [k, n]
    b_eff = (lq * np.einsum("nhb,b->n", Wo_r, bv64) + bo64).astype(np.float32)

    # wc[j*P + p, k*P + c] = W_eff[k*P + p, j*P + c]  (lhsT blocks, natural)
    wc = np.ascontiguousarray(
        W_eff.reshape(KT, P, JT, P).transpose(2, 1, 0, 3).reshape(JT * P, E)
    ).astype(np_dtype)
    bw_blk = np.ascontiguousarray(b_eff.reshape(JT, P).T)  # [p, j] fp32

    wmap = {
        f"w{j}": np.ascontiguousarray(wc[j * P:(j + 1) * P, :])
        for j in range(1, JT)
    }

    X = V.reshape(ROWS, E).astype(np_dtype)
    in_maps = []
    for i in range(N_CORES):
        xs_i = X[i * RPC:(i + 1) * RPC, :].T.reshape(KT, P, RPC)
        m = dict(wmap)
        # chunk layout: X^T slab c occupies columns c*RPC..(c+1)*RPC
        xa = xs_i[0:4].transpose(1, 0, 2).reshape(P, 4 * RPC)
        xb = xs_i[4:8].transpose(1, 0, 2).reshape(P, 4 * RPC)
        wxa_i = np.empty((P, E // 2 + 4 * RPC), np_dtype)
        wxa_i[:, :E // 2] = wc[0:P, 0:E // 2]
        wxa_i[:, E // 2:] = xa
        m["wxa"] = wxa_i
        m["w0b"] = np.ascontiguousarray(wc[0:P, E // 2:])
        m["xbw"] = np.ascontiguousarray(xb[:, 0:3 * RPC])
        m["x7"] = np.ascontiguousarray(xb[:, 3 * RPC:])
        m["bw"] = bw_blk
        in_maps.append(m)
    return in_maps


def kernel(Q, K, V, Wq, bq, Wk, bk, Wv, bv, Wo, bo, dtype_name="bfloat16",
           n_warm=N_WARM, **_unused):
    global LAST_RESULTS
    n, L, e = np.asarray(V).shape
    lq = float(np.asarray(Q).shape[1])
    np_dtype = (np.dtype(ml_dtypes.bfloat16) if dtype_name == "bfloat16"
                else np.float32)
    in_maps = _prep_in_maps(V, Wv, bv, Wo, bo, lq, np_dtype)
    nc = _get_nc(dtype_name, n_warm)
    LAST_RESULTS = run_bass_kernel_spmd(nc, in_maps, list(range(N_CORES)))
    out = np.concatenate(
        [LAST_RESULTS.results[i]["outT"].T.astype(np.float32)
         for i in range(N_CORES)],
        axis=0,
    )
    return np.ascontiguousarray(out).reshape(n, L, E)


# revision 92
# speedup vs baseline: 1.0483x; 1.0208x over previous
"""MultiHeadAttention kernel for 8x TRN2 NeuronCores.

The reference module's einsum reduces the attention tensor over BOTH the
query and key axes (attn_mass = sum_{q,k} softmax(logits)_k), and softmax
rows sum to 1, so attn_mass == Lq exactly for every (batch, head). The
whole computation therefore collapses to

    out = (Lq * (V_heads @ Wv^T + bv)).reshape(N, L, E) @ Wo^T + bo

which is a single dense GEMM after folding the (block-diagonal) per-head
V-projection into the output projection:

    out = V_flat @ W_eff + b_eff
    W_eff[h*hd+a, n] = Lq * sum_b Wv[b, a] * Wo[n, h*hd+b]      (1024 x 1024)
    b_eff[n]         = Lq * sum_{h,b} Wo[n, h*hd+b] * bv[b] + bo[n]

The device kernel is the GEMM in bf16 (the correctness gate is 2e-2
rel-err; bf16 lands ~2.6e-3), row-sharded across 8 cores (512 rows per
core), computed in TRANSPOSED orientation: out^T[n, m] = sum_k W[k, n]
X[m, k].  Each PSUM bank j holds output columns j*128..(j+1)*128 on
partitions x all 512 rows on the free dim, accumulating lhsT = W-block j
(natural layout) against rhs = X^T k-slabs.

Schedule (from NTFF trace analysis; measured constants in comments):
  * fixed costs per launch: ~6.5us BSP/framework preamble before user
    DMAs can issue, ~1.5-2.5us DMA-queue startup after issue, and a
    ~8.8us NEFF close (walrus's 253-semaphore reset storm + final
    barrier) -- none of it controllable from kernel code;
  * matmul full rate is 216ns per [128x128]x[128x512]; the PE p-state
    reaches 2.4 GHz only after ~3-4us of UNINTERRUPTED activity and
    a stall resets it, so short [128,128] warm-up matmuls on memset
    data bridge the preamble -> first-data gap exactly;
  * only the sync + scalar HWDGE queues are used (gpsimd's soft-DGE
    queue collapses HW-queue throughput while active); per-queue rate
    ramps from ~100-250 B/ns to 300-390 B/ns with >=2 KiB lines;
  * the PE gate (w0 k-slabs 0-3 + X slabs 0-3, 0.625 MB) is ONE fused
    transfer leading the sync queue; X slabs 4-7 follow on the same
    queue so their arrival (T0+1.7us) tracks bank 1's k4 deadline
    (T0+1.73us) at any uniform rate; w1 leads the scalar queue and
    lands before T0; all later W blocks have multi-us margin;
  * emission: bank 0's k0-3 partial first (smallest gate), banks 1-7
    full (evenly-spaced completions whose evictions + output DMAs
    overlap the stream), then bank 0's k4-7 finish in a 3/4 chunk
    plus a 1/4-wide full-k chunk that BORROWS bank 1's long-evicted
    PSUM, so the true tail is one small eviction + a 32 KB DMA;
  * bias varies along PARTITIONS -> folded into the PSUM eviction as a
    per-partition tensor_scalar_add on the vector engine, which also
    casts fp32 PSUM to the bf16 output (separate o tiles per chunk --
    a shared tile makes Tile serialize behind output-DMA delivery).

The host transposes V-shards in, and the (E, RPC) per-core outputs back.
"""

import numpy as np
import ml_dtypes

import concourse.bass as bass
import concourse.bacc as bacc
import concourse.mybir as mybir
from concourse.tile import TileContext
from concourse.bass_utils import run_bass_kernel_spmd

N_CORES = 8
E = 1024            # embed dim == d_model
H, HD = 16, 64      # heads, head dim
ROWS = 4096         # N * L = 2 * 2048
RPC = ROWS // N_CORES   # rows per core = 512
P = 128             # SBUF partitions
KT = E // P         # 8 contraction slabs
JT = E // P         # 8 output-column banks
N_WARM = 37         # bf16 warm-up matmuls bridging preamble -> first data

_NC_CACHE = {}
LAST_RESULTS = None  # BassKernelResults of the most recent device run


def _build(dtype, n_warm=N_WARM):
    f32 = mybir.dt.float32
    odt = f32 if dtype == mybir.dt.float32r else dtype
    nc = bacc.Bacc(None, target_bir_lowering=False)
    # PE-gating data fused into ONE DMA per queue (one issue slot, one
    # completion semaphore): wxa = [w0 | X^T slabs 0-3] on sync,
    # xbw = [X^T slabs 4-7 | bias(bf16)] on scalar.  Fat >=4 KiB lines
    # run ~300+ GB/s/queue once the DMA path has ramped.
    wxa = nc.declare_dram_parameter("wxa", [P, E // 2 + 4 * RPC], dtype,
                                    isOutput=False)
    w0b = nc.declare_dram_parameter("w0b", [P, E // 2], dtype, isOutput=False)
    xbw = nc.declare_dram_parameter("xbw", [P, 3 * RPC], dtype,
                                    isOutput=False)
    x7 = nc.declare_dram_parameter("x7", [P, RPC], dtype, isOutput=False)
    bw = nc.declare_dram_parameter("bw", [P, JT], f32, isOutput=False)
    w1a = nc.declare_dram_parameter("w1a", [P, E // 2], dtype, isOutput=False)
    w1b = nc.declare_dram_parameter("w1b", [P, E // 2], dtype, isOutput=False)
    ws = [None, None] + [
        nc.declare_dram_parameter(f"w{j}", [P, E], dtype, isOutput=False)
        for j in range(2, JT)
    ]
    outT = nc.declare_dram_parameter("outT", [E, RPC], odt, isOutput=True)

    with TileContext(nc) as tc:
        with (
            tc.tile_pool(name="xp", bufs=1) as xp,
            tc.tile_pool(name="wp", bufs=1) as wp,
            tc.tile_pool(name="bp", bufs=1) as bp,
            tc.tile_pool(name="pp", bufs=1, space="PSUM") as pp,
            tc.tile_pool(name="op", bufs=1) as op,
        ):
            # memset needs no DMA: warm-up matmuls can start right after
            # the BSP preamble, well before any input data lands.
            wm_t = bp.tile([P, P], dtype, name="wm", tag="wm")
            nc.gpsimd.memset(wm_t[:], 1.0)

            wxa_t = xp.tile([P, E // 2 + 4 * RPC], dtype, name="wxa",
                            tag="wxa")
            w0b_t = xp.tile([P, E // 2], dtype, name="w0b", tag="w0b")
            xbw_t = xp.tile([P, 3 * RPC], dtype, name="xbw", tag="xbw")
            x7_t = xp.tile([P, RPC], dtype, name="x7", tag="x7")
            bw_t = bp.tile([P, JT], f32, name="bw", tag="bw")
            w1a_t = wp.tile([P, E // 2], dtype, name="w1a", tag="w1a")
            w1b_t = wp.tile([P, E // 2], dtype, name="w1b", tag="w1b")
            wts = [None, None] + [
                wp.tile([P, E], dtype, name=f"w{j}", tag=f"w{j}")
                for j in range(2, JT)
            ]

            def w0s(k):
                if k < 4:
                    return wxa_t[:, k * P:(k + 1) * P]
                return w0b_t[:, (k - 4) * P:(k - 3) * P]

            def lhs(j, k):
                if j == 0:
                    return w0s(k)
                if j == 1:
                    t = w1a_t if k < 4 else w1b_t
                    return t[:, (k % 4) * P:(k % 4 + 1) * P]
                return wts[j][:, k * P:(k + 1) * P]

            def bias(j):
                return bw_t[:, j:j + 1]

            # Queue plan.  Measured behavior: the DMA path ramps over the
            # first ~3us (both HW queues ~100-250 B/ns early, 300-390
            # after); first packets ~1.5-2.5us after issue; each
            # dma_start costs ~0.6-0.8us of issue time on its engine.
            # gpsimd's software-DGE queue STRANGLES the HW queues while
            # active (measured 3x collapse) -- never use it alongside
            # the critical stream.
            # Rate-correlated feed: the PE-gating wxa (w0 k-slabs 0-3 +
            # X slabs 0-3, 0.625 MB) and then xbw (X slabs 4-7) both
            # ride the EARLIER-starting sync queue back to back.  Banks
            # 0-3 all run k0-3 first, so xbw's deadline is T0 + 3.46us
            # against an arrival of T0 + ~1.7-3us even on slow runs.
            # w1-w3 lead the scalar queue (deadlines T0+0.86/1.73/2.6).
            # X slab 7 peels off to the scalar queue so the sync-queue
            # chase (xbw = slabs 4-6 only) lands at T0+1.25 against its
            # T0+1.73 deadline, while slab 7's deadline is T0+2.4
            # against a ~T0 arrival on scalar.
            #   sync:   wxa, xbw(4-6), w0b, w5, w7  (+ even outputs)
            #   scalar: w1, x7, bw, w2, w3, w4, w6  (+ odd outputs)
            nc.sync.dma_start(out=wxa_t[:], in_=wxa[:, :])
            nc.scalar.dma_start(out=w1a_t[:], in_=w1a[:, :])
            nc.scalar.dma_start(out=w1b_t[:], in_=w1b[:, :])
            nc.sync.dma_start(out=xbw_t[:], in_=xbw[:, :])
            nc.scalar.dma_start(out=x7_t[:], in_=x7[:, :])
            nc.scalar.dma_start(out=bw_t[:], in_=bw[:, :])
            nc.scalar.dma_start(out=wts[2][:], in_=ws[2][:, :])
            nc.sync.dma_start(out=w0b_t[:], in_=w0b[:, :])
            nc.scalar.dma_start(out=wts[3][:], in_=ws[3][:, :])
            nc.scalar.dma_start(out=wts[4][:], in_=ws[4][:, :])
            nc.sync.dma_start(out=wts[5][:], in_=ws[5][:, :])
            nc.scalar.dma_start(out=wts[6][:], in_=ws[6][:, :])
            nc.sync.dma_start(out=wts[7][:], in_=ws[7][:, :])

            q3 = 3 * RPC // 4
            ps = [
                pp.tile([P, RPC], f32, name=f"ps{j}", tag=f"ps{j}")
                for j in range(JT)
            ]

            # Low-activity bf16 PE warm-up on nonzero memset data,
            # bridging the preamble -> first-data gap so the PE stays
            # continuously busy (p-state ramps to 2.4 GHz only after
            # ~3-4us of uninterrupted activity; a stall resets it).
            # NOTE: full-width warm-ups measured ~4us SLOWER overall
            # (HAM activity budget) -- keep them 128 wide.
            for i in range(n_warm):
                nc.tensor.matmul(
                    ps[i % JT][:, 0:P],
                    wm_t[:, 0:P],
                    wm_t[:, 0:P],
                    start=True,
                    stop=True,
                )

            def rhs(k):
                if k < 4:
                    return wxa_t[:, E // 2 + k * RPC:E // 2 + (k + 1) * RPC]
                if k < 7:
                    return xbw_t[:, (k - 4) * RPC:(k - 3) * RPC]
                return x7_t[:, :]

            # Emission order: bank 0's k0-3 partial first (its gate is
            # only the 0.625 MB wxa transfer), then banks 1-7 in full,
            # then bank 0's k4-7 finish.  Banks complete ~evenly spread
            # so their evictions + output DMAs overlap the stream; the
            # LAST eviction is split 3/4 + 1/4 with SEPARATE o tiles (a
            # shared tile makes Tile serialize the second eviction
            # behind the first chunk's output DMA delivery).
            def emit(j, ks, stop_at_end):
                for k in ks:
                    nc.tensor.matmul(
                        ps[j] if j != 0 else ps[0][:, 0:q3],
                        lhs(j, k),
                        rhs(k) if j != 0 else rhs(k)[:, 0:q3],
                        start=(k == 0),
                        stop=(stop_at_end and k == ks[-1]),
                    )

            def evict(j):
                o = op.tile([P, RPC], odt, name=f"o{j}", tag=f"o{j}")
                oq = nc.sync if j % 2 == 0 else nc.scalar
                nc.vector.tensor_scalar_add(o[:], ps[j], bias(j))
                oq.dma_start(out=outT[j * P:(j + 1) * P, :], in_=o[:])

            emit(0, list(range(4)), stop_at_end=False)
            for j in range(1, JT):
                emit(j, list(range(KT)), stop_at_end=True)
                evict(j)
            # bank 0 chunk A (cols 0:q3): finish k4-7 and evict.
            for k in range(4, KT):
                nc.tensor.matmul(
                    ps[0][:, 0:q3],
                    w0s(k),
                    rhs(k)[:, 0:q3],
                    start=False,
                    stop=(k == KT - 1),
                )
            oa = op.tile([P, q3], odt, name="o0a", tag="o0a")
            nc.vector.tensor_scalar_add(oa[:], ps[0][:, 0:q3], bias(0))
            nc.sync.dma_start(out=outT[0:P, 0:q3], in_=oa[:])
            # bank 0 chunk B (last RPC-q3 cols): full k0-7 accumulation
            # at the very end, BORROWING bank 1's psum (long evicted).
            # Its matmuls overlap chunk A's eviction; the small final
            # eviction + 32 KB output DMA are the only true tail.
            for k in range(KT):
                nc.tensor.matmul(
                    ps[1][:, 0:RPC - q3],
                    w0s(k),
                    rhs(k)[:, q3:RPC],
                    start=(k == 0),
                    stop=(k == KT - 1),
                )
            ob = op.tile([P, RPC - q3], odt, name="o0b", tag="o0b")
            nc.vector.tensor_scalar_add(ob[:], ps[1][:, 0:RPC - q3], bias(0))
            # single final 32 KB output on scalar (whose engine is free
            # the moment the eviction lands; sync is still issuing
            # chunk A's output then -- a split across both queues
            # measured ~0.6us SLOWER).
            nc.scalar.dma_start(out=outT[0:P, q3:RPC], in_=ob[:, :])
    nc.compile()
    return nc


def _get_nc(dtype_name, n_warm=N_WARM):
    key = (dtype_name, n_warm)
    if key not in _NC_CACHE:
        _NC_CACHE[key] = _build(getattr(mybir.dt, dtype_name), n_warm)
    return _NC_CACHE[key]


def _prep_in_maps(V, Wv, bv, Wo, bo, lq, np_dtype):
    V = np.ascontiguousarray(np.asarray(V, dtype=np.float32))
    Wv64 = np.asarray(Wv, np.float64)
    Wo64 = np.asarray(Wo, np.float64)
    bv64 = np.asarray(bv, np.float64)
    bo64 = np.asarray(bo, np.float64)

    # Fold per-head V-projection + output projection + attention mass (== Lq).
    Wo_r = Wo64.reshape(E, H, HD)                       # [n, h, b]
    W_eff = lq * np.einsum("ba,nhb->han", Wv64, Wo_r, optimize=True)
    W_eff = W_eff.reshape(E, E).astype(np.float32)      # [k, n]
    b_eff = (lq * np.einsum("nhb,b->n", Wo_r, bv64) + bo64).astype(np.float32)

    # wc[j*P + p, k*P + c] = W_eff[k*P + p, j*P + c]  (lhsT blocks, natural)
    wc = np.ascontiguousarray(
        W_eff.reshape(KT, P, JT, P).transpose(2, 1, 0, 3).reshape(JT * P, E)
    ).astype(np_dtype)
    bw_blk = np.ascontiguousarray(b_eff.reshape(JT, P).T)  # [p, j] fp32

    wmap = {
        f"w{j}": np.ascontiguousarray(wc[j * P:(j + 1) * P, :])
        for j in range(2, JT)
    }
    wmap["w1a"] = np.ascontiguousarray(wc[P:2 * P, 0:E // 2])
    wmap["w1b"] = np.ascontiguousarray(wc[P:2 * P, E // 2:])

    X = V.reshape(ROWS, E).astype(np_dtype)
    in_maps = []
    for i in range(N_CORES):
        xs_i = X[i * RPC:(i + 1) * RPC, :].T.reshape(KT, P, RPC)
        m = dict(wmap)
        # chunk layout: X^T slab c occupies columns c*RPC..(c+1)*RPC
        xa = xs_i[0:4].transpose(1, 0, 2).reshape(P, 4 * RPC)
        xb = xs_i[4:8].transpose(1, 0, 2).reshape(P, 4 * RPC)
        wxa_i = np.empty((P, E // 2 + 4 * RPC), np_dtype)
        wxa_i[:, :E // 2] = wc[0:P, 0:E // 2]
        wxa_i[:, E // 2:] = xa
        m["wxa"] = wxa_i
        m["w0b"] = np.ascontiguousarray(wc[0:P, E // 2:])
        m["xbw"] = np.ascontiguousarray(xb[:, 0:3 * RPC])
        m["x7"] = np.ascontiguousarray(xb[:, 3 * RPC:])
        m["bw"] = bw_blk
        in_maps.append(m)
    return in_maps


def kernel(Q, K, V, Wq, bq, Wk, bk, Wv, bv, Wo, bo, dtype_name="bfloat16",
           n_warm=N_WARM, **_unused):
    global LAST_RESULTS
    n, L, e = np.asarray(V).shape
    lq = float(np.asarray(Q).shape[1])
    np_dtype = (np.dtype(ml_dtypes.bfloat16) if dtype_name == "bfloat16"
                else np.float32)
    in_maps = _prep_in_maps(V, Wv, bv, Wo, bo, lq, np_dtype)
    nc = _get_nc(dtype_name, n_warm)
    LAST_RESULTS = run_bass_kernel_spmd(nc, in_maps, list(range(N_CORES)))
    out = np.concatenate(
        [LAST_RESULTS.results[i]["outT"].T.astype(np.float32)
         for i in range(N_CORES)],
        axis=0,
    )
    return np.ascontiguousarray(out).reshape(n, L, E)
